# revision 1
# baseline (speedup 1.0000x reference)
"""Trainium2 Bass kernel for the hardest-positive triplet-softplus loss.

Strategy (data-parallel over distance-matrix rows, 8 NeuronCores):
  - Each core owns a 512-row block of the 4096-row pairwise structure.
  - Selection matrix in fp16 on the PE at full rate:
        S[i,j] = 2*dot(x_i,x_j) + BIG*same(i,j) - (sq_j - 512)
    built from a 640-dim extended contraction ([2x; onehot] x [x; BIG*onehot])
    plus a K=1 matmul adding the centered -sq_j term, and a -2*BIG additive
    mask knocking out the diagonal.  Row-wise argmax of S picks the hardest
    positive (min distance); per-core column rotation keeps the diagonal in
    column-block 0 so the program stays SPMD.
  - Argmax is extracted with a fused one-pass trick per PSUM tile:
    reduce_max gives the tile max, then ((S >= max) * iota) summed via
    scalar_tensor_tensor's accumulator yields the argmax column; a second
    application of the same trick across the 8 tile partials selects the
    global winner.  No PSUM->SBUF spill of the matrix is ever needed.
  - The winning rows are fetched with an indirect DMA gather from a
    per-core pre-rolled copy of the batch; d(a,p) and d(a,n) are then
    recomputed exactly in fp32 (sub -> square-accumulate), matching the
    reference formulation bit-for-bit in spirit.
  - Per-row stable-softplus tail on scalar/vector engines; each core returns
    a [128,1] per-partition partial sum, combined (and divided by the
    host-known valid count) on the host -- that is the unshard step.
"""

import os
import sys

import numpy as np

for _p in ("/opt/trn_rl_repo", "/root/.axon_site/_ro/trn_rl_repo"):
    if os.path.isdir(_p) and _p not in sys.path:
        sys.path.append(_p)

import concourse.bass as bass  # noqa: E402
import concourse.bacc as bacc  # noqa: E402
import concourse.tile as tile  # noqa: E402
from concourse import mybir  # noqa: E402
from concourse import bass_utils  # noqa: E402

B = 4096
DIM = 512
C = 128
NCORES = 8
RB = B // NCORES          # rows per core
NK = (DIM + C) // 128     # 5 contraction tiles of 128
NM = RB // 128            # 4 row tiles per core
NN = B // 512             # 8 column blocks of 512
BIG = 4096.0
EPS = 1e-12

F32 = mybir.dt.float32
F16 = mybir.dt.float16
U32 = mybir.dt.uint32
ALU = mybir.AluOpType
AFT = mybir.ActivationFunctionType
AX = mybir.AxisListType

# rhs column blocks are consumed (and DMAed) in this order; the diagonal block
# (rolled position 0) goes last so its mask DMA can trail the first chunks.
N_ORDER = [1, 2, 3, 4, 5, 6, 7, 0]

_NC_CACHE = None


def _build_nc():
    nc = bacc.Bacc(
        "TRN2",
        target_bir_lowering=False,
        debug=False,
        enable_asserts=False,
    )

    rhs_d = nc.dram_tensor("rhs", [NN, 128, NK * 512], F16, kind="ExternalInput").ap()
    lhs_d = nc.dram_tensor("lhsx", [128, NK * 512], F16, kind="ExternalInput").ap()
    sqcn_d = nc.dram_tensor("sqcn", [1, B], F16, kind="ExternalInput").ap()
    diag_d = nc.dram_tensor("diagm", [128, NM * 512], F16, kind="ExternalInput").ap()
    iota_d = nc.dram_tensor("iotam", [128, B], F32, kind="ExternalInput").ap()
    bt_d = nc.dram_tensor("batcht", [B, DIM], F32, kind="ExternalInput").ap()
    xrow_d = nc.dram_tensor("xrow", [128, NM * 512], F32, kind="ExternalInput").ap()
    xneg_d = nc.dram_tensor("xneg", [128, NM * 512], F32, kind="ExternalInput").ap()
    vld_d = nc.dram_tensor("vld", [128, NM], F32, kind="ExternalInput").ap()
    out_d = nc.dram_tensor("out", [128, 1], F32, kind="ExternalOutput").ap()

    with tile.TileContext(nc) as tc:
        with (
            tc.tile_pool(name="big", bufs=1) as big,
            tc.tile_pool(name="work", bufs=4) as work,
            tc.tile_pool(name="ps", bufs=6, space="PSUM") as pp,
            tc.tile_pool(name="sm", bufs=1) as sm,
        ):
            lhs_sb = big.tile([128, NK * 512], F16, tag="lhs")
            nc.sync.dma_start(lhs_sb[:], lhs_d[:])
            sqcn_sb = big.tile([1, B], F16, tag="sqcn")
            nc.sync.dma_start(sqcn_sb[:], sqcn_d[:])
            ones1 = sm.tile([1, 128], F16, tag="ones1")
            nc.vector.memset(ones1[:], 1.0)

            rhs_sb = {}
            for i, n in enumerate(N_ORDER):
                t = big.tile([128, NK * 512], F16, tag=f"rhs{n}", name=f"rhs{n}")
                nc.sync.dma_start(t[:], rhs_d[n])
                rhs_sb[n] = t
                if i == 0:
                    diag_sb = big.tile([128, NM * 512], F16, tag="diag", name="diag")
                    nc.sync.dma_start(diag_sb[:], diag_d[:])
                elif i == 1:
                    iota_sb = big.tile([128, B], F32, tag="iota", name="iota")
                    nc.sync.dma_start(iota_sb[:], iota_d[:])
                elif i == 3:
                    xr_sb = big.tile([128, NM * 512], F32, tag="xr", name="xr")
                    nc.sync.dma_start(xr_sb[:], xrow_d[:])
                    xn_sb = big.tile([128, NM * 512], F32, tag="xn", name="xn")
                    nc.sync.dma_start(xn_sb[:], xneg_d[:])

            vld = sm.tile([128, NM], F32, tag="vld")
            nc.sync.dma_start(vld[:], vld_d[:])
            epsb = sm.tile([128, 1], F32, tag="epsb")
            nc.gpsimd.memset(epsb[:], EPS)

            d2ap = sm.tile([128, NM], F32, tag="d2ap")
            d2an = sm.tile([128, NM], F32, tag="d2an")
            parts = [
                sm.tile([128, NN], F32, tag=f"parts{m}", name=f"parts{m}")
                for m in range(NM)
            ]
            ixparts = [
                sm.tile([128, NN], F32, tag=f"ixparts{m}", name=f"ixparts{m}")
                for m in range(NM)
            ]

            # ---- d(a, negative): exact fp32, independent of the matrix ----
            for m in range(NM):
                ms = slice(m * 512, (m + 1) * 512)
                dsc = work.tile([128, DIM], F32, tag="dsc", name="dsc")
                nc.vector.tensor_sub(dsc[:], xr_sb[:, ms], xn_sb[:, ms])
                ssc = work.tile([128, DIM], F32, tag="ssc", name="ssc")
                nc.scalar.activation(
                    ssc[:], dsc[:], AFT.Square, accum_out=d2an[:, m:m + 1]
                )

            # ---- main pass: n-outer keeps the PE dense behind the DMA ----
            for i, n in enumerate(N_ORDER):
                for m in range(NM):
                    pt = pp.tile([128, 512], F32, tag="acc", name="acc")
                    for k in range(NK):
                        nc.tensor.matmul(
                            pt[:],
                            lhs_sb[:, k * 512 + m * 128:k * 512 + (m + 1) * 128],
                            rhs_sb[n][:, k * 512:(k + 1) * 512],
                            start=(k == 0),
                            stop=False,
                        )
                    # centered -sq_j via a K=1 matmul into the same psum group
                    nc.tensor.matmul(
                        pt[:],
                        ones1[:],
                        sqcn_sb[:, n * 512:(n + 1) * 512],
                        start=False,
                        stop=True,
                    )
                    if n == 0:
                        nc.vector.tensor_add(
                            pt[:], pt[:], diag_sb[:, m * 512:(m + 1) * 512]
                        )
                    # per-tile max + fused argmax ((S>=max)*iota summed)
                    nc.vector.tensor_reduce(
                        parts[m][:, i:i + 1], pt[:], axis=AX.X, op=ALU.max
                    )
                    junk = work.tile([128, 512], F32, tag="junk", name="junk")
                    nc.vector.scalar_tensor_tensor(
                        junk[:], pt[:], parts[m][:, i:i + 1],
                        iota_sb[:, n * 512:(n + 1) * 512],
                        op0=ALU.is_ge, op1=ALU.mult,
                        accum_out=ixparts[m][:, i:i + 1],
                    )

                    if i == NN - 1:
                        # last column block: finalize this m-tile
                        mxv = work.tile([128, 1], F32, tag="mxv", name="mxv")
                        nc.vector.tensor_reduce(
                            mxv[:], parts[m][:], axis=AX.X, op=ALU.max
                        )
                        junk8 = work.tile([128, NN], F32, tag="junk8", name="junk8")
                        idxf = work.tile([128, 1], F32, tag="idxf", name="idxf")
                        nc.vector.scalar_tensor_tensor(
                            junk8[:], parts[m][:], mxv[:], ixparts[m][:],
                            op0=ALU.is_ge, op1=ALU.mult, accum_out=idxf[:],
                        )
                        nc.vector.tensor_scalar(
                            idxf[:], idxf[:], float(B - 1), None, op0=ALU.min
                        )
                        idxu = work.tile([128, 1], U32, tag="idxu", name="idxu")
                        nc.vector.tensor_copy(idxu[:], idxf[:])
                        xp = work.tile([128, DIM], F32, tag="xp", name="xp")
                        nc.gpsimd.indirect_dma_start(
                            out=xp[:], out_offset=None, in_=bt_d[:],
                            in_offset=bass.IndirectOffsetOnAxis(
                                ap=idxu[:, :1], axis=0),
                        )
                        ms = slice(m * 512, (m + 1) * 512)
                        dsc = work.tile([128, DIM], F32, tag="dsc", name="dsc")
                        nc.vector.tensor_sub(dsc[:], xr_sb[:, ms], xp[:])
                        ssc = work.tile([128, DIM], F32, tag="ssc", name="ssc")
                        nc.scalar.activation(
                            ssc[:], dsc[:], AFT.Square,
                            accum_out=d2ap[:, m:m + 1],
                        )

            # ---- per-row tail ([128, 4] tensors) ----
            # sqrt(max(d2,eps)) == sqrt(d2+eps) in fp32 for d2 >= 0
            dap = sm.tile([128, NM], F32, tag="dap")
            nc.scalar.activation(dap[:], d2ap[:], AFT.Sqrt, bias=epsb[:])
            dan = sm.tile([128, NM], F32, tag="dan")
            nc.scalar.activation(dan[:], d2an[:], AFT.Sqrt, bias=epsb[:])
            zd = sm.tile([128, NM], F32, tag="zd")
            nc.vector.tensor_sub(zd[:], dap[:], dan[:])
            a1 = sm.tile([128, NM], F32, tag="a1")
            nc.scalar.activation(a1[:], zd[:], AFT.Relu, scale=10.0)
            a2 = sm.tile([128, NM], F32, tag="a2")
            nc.scalar.activation(a2[:], zd[:], AFT.Relu, scale=-10.0)
            s = sm.tile([128, NM], F32, tag="s")
            nc.vector.tensor_add(s[:], a1[:], a2[:])            # |10*zd|
            e = sm.tile([128, NM], F32, tag="e")
            nc.scalar.activation(e[:], s[:], AFT.Exp, scale=-1.0)
            ln1p = sm.tile([128, NM], F32, tag="ln1p")
            nc.scalar.activation(ln1p[:], e[:], AFT.Ln, bias=1.0)
            per = sm.tile([128, NM], F32, tag="per")
            nc.vector.tensor_add(per[:], a1[:], ln1p[:])        # softplus(10*zd)
            w = sm.tile([128, NM], F32, tag="w")
            nc.vector.tensor_mul(w[:], per[:], vld[:])
            prt = sm.tile([128, 1], F32, tag="prt")
            nc.vector.tensor_reduce(prt[:], w[:], axis=AX.X, op=ALU.add)
            nc.sync.dma_start(out_d[:], prt[:])

    nc.compile()
    return nc


def get_nc():
    global _NC_CACHE
    if _NC_CACHE is None:
        _NC_CACHE = _build_nc()
    return _NC_CACHE


def _prep_inputs(batch, labels, anchors, negatives):
    """Host-side sharding prep: build the 8 per-core input maps."""
    batch = np.ascontiguousarray(np.asarray(batch), dtype=np.float32)
    labels = np.asarray(labels).astype(np.int64)
    anchors = np.asarray(anchors).astype(np.int64)
    negatives = np.asarray(negatives).astype(np.int64)

    sq = (batch * batch).sum(axis=1, dtype=np.float32)          # [B]
    onehotT = np.zeros((C, B), np.float32)
    onehotT[labels, np.arange(B)] = 1.0

    rhs_full = np.empty((NK * 128, B), np.float16)
    rhs_full[:DIM] = batch.T
    rhs_full[DIM:] = BIG * onehotT
    # [n, p, k*512+q] layout: per-n chunks are single contiguous DMAs
    rhs_chunks = np.ascontiguousarray(
        rhs_full.reshape(NK, 128, NN, 512).transpose(2, 1, 0, 3).reshape(
            NN, 128, NK * 512)
    )
    sqcn = -(sq - np.float32(512.0)).astype(np.float16)         # [B]

    diag = np.zeros((128, NM * 512), np.float16)
    p = np.arange(128)
    for m in range(NM):
        diag[p, 512 * m + 128 * m + p] = -2.0 * BIG

    iota = np.broadcast_to(
        np.arange(B, dtype=np.float32), (128, B)).copy()

    hist = np.bincount(labels, minlength=C)
    valid = (hist[labels] - 1) > 1                              # [B] bool
    count = float(valid.sum())

    in_maps = []
    for c in range(NCORES):
        r0 = c * RB
        rows = slice(r0, r0 + RB)
        arow = anchors[rows]
        nrow = negatives[rows]

        lhs = np.empty((NK * 128, RB), np.float16)
        lhs[:DIM] = 2.0 * batch[rows].T
        lhs[DIM:] = onehotT[:, rows]

        perm = (np.arange(NN) + c) % NN
        in_maps.append({
            "rhs": np.ascontiguousarray(rhs_chunks[perm]),
            "lhsx": np.ascontiguousarray(
                lhs.reshape(NK, 128, RB).transpose(1, 0, 2).reshape(
                    128, NK * 512)),
            "sqcn": np.ascontiguousarray(np.roll(sqcn, -r0)[None, :]),
            "diagm": diag,
            "iotam": iota,
            "batcht": np.roll(batch, -r0, axis=0),
            "xrow": np.ascontiguousarray(
                batch[arow].reshape(NM, 128, DIM).transpose(1, 0, 2).reshape(
                    128, NM * 512)),
            "xneg": np.ascontiguousarray(
                batch[nrow].reshape(NM, 128, DIM).transpose(1, 0, 2).reshape(
                    128, NM * 512)),
            "vld": np.ascontiguousarray(
                valid[rows].astype(np.float32).reshape(NM, 128).T),
        })
    return in_maps, count


def kernel(batch, labels, anchors, negatives, **_kwargs):
    in_maps, count = _prep_inputs(batch, labels, anchors, negatives)
    nc = get_nc()
    res = bass_utils.run_bass_kernel_spmd(nc, in_maps, core_ids=list(range(NCORES)))
    total = sum(r["out"].sum(dtype=np.float64) for r in res.results)
    loss = np.float32(np.float32(total) / np.float32(count))
    return np.array([loss], dtype=np.float32)



# revision 4
# speedup vs baseline: 4.9622x; 4.9622x over previous
"""Trainium2 Bass kernel for the hardest-positive triplet-softplus loss.

Key observation: the reference builds the full 4096x4096 distance matrix but
only ever *uses* same-label entries (hardest-positive mining per row).  With
C=128 classes over B=4096 rows, each class has ~32 members.  Sorting rows by
label on the host makes every row's positives live in a small contiguous band
of the sorted order.

Strategy (8 NeuronCores, data-parallel over sorted row windows):
  - Host sorts rows by label (stable).  Each core owns 4 windows of 128
    consecutive sorted rows.  For window w starting at sorted position `base`,
    every positive of every row in the window lies within sorted positions
    [base-64, base+192)  (class size <= ~51 for this distribution; asserted).
  - Device computes, per window, the [128 x 256] Gram block
        G[p, j] = dot(x_row(base+p), x_col(base-64+j))
    via 4 accumulating fp16 matmuls (K=512).  The lhsT operand is just the
    center 128 columns of the same SBUF tile -- no separate lhs upload.
  - A single fused DVE op (tensor_tensor_reduce) adds a host-built additive
    mask tile and takes the row max in one pass:
        mask[p, j] = 256 - sq_col/2   if same label, not self, in range
                   = -30000           otherwise
        v[p] = max_j (G[p, j] + mask[p, j])
    Winner value v encodes the hardest-positive distance exactly:
        d2_ap = sq_row + 512 - 2*v        (recovered on host)
  - Everything else is exact host numpy: d_an from the raw fp32 batch,
    softplus tail, valid mask / count, final mean.  Device output is just
    [128, 4] fp32 row-max values per core.
"""

import os
import sys

import numpy as np

for _p in ("/opt/trn_rl_repo", "/root/.axon_site/_ro/trn_rl_repo"):
    if os.path.isdir(_p) and _p not in sys.path:
        sys.path.append(_p)

import concourse.bass as bass  # noqa: E402
import concourse.bacc as bacc  # noqa: E402
import concourse.tile as tile  # noqa: E402
from concourse import mybir  # noqa: E402
from concourse import bass_utils  # noqa: E402

B = 4096
DIM = 512
C = 128
TEMP = 0.05
NCORES = 8
NW = 4            # windows of 128 sorted rows per core
W = 256           # columns per window (center 128 + 64 pad each side)
NK = DIM // 128   # 4 contraction tiles
LHS0 = 64         # offset of the window's own rows inside the 256 columns
NEG = -30000.0    # mask value for non-positive columns

F32 = mybir.dt.float32
F16 = mybir.dt.float16
ALU = mybir.AluOpType
AX = mybir.AxisListType

_NC_CACHE = None


def _build_nc():
    nc = bacc.Bacc(
        "TRN2",
        target_bir_lowering=False,
        debug=False,
        enable_asserts=False,
    )

    rhs_d = nc.dram_tensor("rhsw", [NW, 128, NK, W], F16, kind="ExternalInput").ap()
    msk_d = nc.dram_tensor("maskw", [NW, 128, W], F16, kind="ExternalInput").ap()
    out_d = nc.dram_tensor("out", [128, NW], F32, kind="ExternalOutput").ap()

    with tile.TileContext(nc) as tc:
        with (
            tc.tile_pool(name="big", bufs=1) as big,
            tc.tile_pool(name="ps", bufs=4, space="PSUM") as pp,
            tc.tile_pool(name="sm", bufs=1) as sm,
        ):
            rhs_sb = []
            msk_sb = []
            for w in range(NW):
                t = big.tile([128, NK, W], F16, tag=f"rhs{w}", name=f"rhs{w}")
                nc.sync.dma_start(t[:], rhs_d[w])
                rhs_sb.append(t)
                mt = big.tile([128, W], F16, tag=f"msk{w}", name=f"msk{w}")
                nc.sync.dma_start(mt[:], msk_d[w])
                msk_sb.append(mt)

            outt = sm.tile([128, NW], F32, tag="outt")
            junk = sm.tile([128, W], F32, tag="junk")

            for w in range(NW):
                pt = pp.tile([128, W], F32, tag="acc", name="acc")
                for k in range(NK):
                    nc.tensor.matmul(
                        pt[:],
                        rhs_sb[w][:, k, LHS0:LHS0 + 128],
                        rhs_sb[w][:, k, :],
                        start=(k == 0),
                        stop=(k == NK - 1),
                    )
                # tensor_tensor_reduce would fuse these, but it wedges the
                # device on TRN2 hardware (sim-only support, apparently).
                nc.vector.tensor_tensor(
                    junk[:], pt[:], msk_sb[w][:], op=ALU.add)
                nc.vector.tensor_reduce(
                    outt[:, w:w + 1], junk[:], axis=AX.X, op=ALU.max)

            nc.sync.dma_start(out_d[:], outt[:])

    nc.compile()
    return nc


def get_nc():
    global _NC_CACHE
    if _NC_CACHE is None:
        _NC_CACHE = _build_nc()
    return _NC_CACHE


def _prep_inputs(batch, labels, anchors=None, negatives=None):
    """Host-side prep: per-core window tensors + (order, sqs) for unshard."""
    batch = np.ascontiguousarray(np.asarray(batch), dtype=np.float32)
    labels = np.asarray(labels).astype(np.int64)

    order = np.argsort(labels, kind="stable").astype(np.int64)
    slab = labels[order]
    xs = batch[order]
    sqs = np.einsum("ij,ij->i", xs, xs, dtype=np.float64)

    xsT = np.ascontiguousarray(xs.T.astype(np.float16))          # [DIM, B]
    maskvals = 256.0 - sqs / 2.0                                  # [B] float64

    # containment: every row's class fits in its window's 256 columns
    starts = np.searchsorted(slab, slab, side="left")
    ends = np.searchsorted(slab, slab, side="right")

    in_maps = []
    for c in range(NCORES):
        rhs = np.empty((NW, 128, NK, W), np.float16)
        msk = np.empty((NW, 128, W), np.float16)
        for wl in range(NW):
            base = (c * NW + wl) * 128
            assert starts[base] >= base - LHS0, "class overflows window left pad"
            assert ends[base + 127] <= base + (W - LHS0), (
                "class overflows window right pad")
            colpos = base - LHS0 + np.arange(W)
            validc = (colpos >= 0) & (colpos < B)
            cp = np.clip(colpos, 0, B - 1)
            rhs[wl] = xsT[:, cp].reshape(NK, 128, W).transpose(1, 0, 2)
            rowpos = base + np.arange(128)
            ok = (validc[None, :]
                  & (slab[cp][None, :] == slab[rowpos][:, None])
                  & (colpos[None, :] != rowpos[:, None]))
            msk[wl] = np.where(ok, maskvals[cp][None, :], NEG).astype(np.float16)
        in_maps.append({"rhsw": rhs, "maskw": msk})
    return in_maps, order, sqs


def kernel(batch, labels, anchors=None, negatives=None, **_kwargs):
    batch = np.ascontiguousarray(np.asarray(batch), dtype=np.float32)
    labels_np = np.asarray(labels).astype(np.int64)
    negatives_np = np.asarray(negatives).astype(np.int64)

    in_maps, order, sqs = _prep_inputs(batch, labels_np)
    nc = get_nc()
    res = bass_utils.run_bass_kernel_spmd(nc, in_maps, core_ids=list(range(NCORES)))

    v = np.stack([np.asarray(r["out"], dtype=np.float64) for r in res.results])
    vsorted = v.transpose(0, 2, 1).reshape(B)     # [core, w, p] -> sorted pos
    d2ap_sorted = sqs + 512.0 - 2.0 * vsorted
    d2_ap = np.empty(B, dtype=np.float64)
    d2_ap[order] = d2ap_sorted
    d_ap = np.sqrt(np.maximum(d2_ap, 1e-12))

    diff = batch.astype(np.float64) - batch[negatives_np].astype(np.float64)
    d_an = np.sqrt(np.maximum(np.einsum("ij,ij->i", diff, diff), 1e-12))

    z = (d_ap - d_an) / (2.0 * TEMP)
    per = np.logaddexp(0.0, z)

    hist = np.bincount(labels_np, minlength=C)
    valid = (hist[labels_np] - 1) > 1
    count = float(valid.sum())
    loss = float((per * valid.astype(np.float64)).sum() / count)
    return np.array([loss], dtype=np.float32)
